# revision 18
# baseline (speedup 1.0000x reference)
"""Discriminative loss kernel for Trainium2 (8 NeuronCores, batch-parallel).

Device kernel (per core, 2 batches): the O(B*L*N) "pull" pass.
  d2[l,n] = e_sq[n] - 2*cross[l,n] + m_sq[l] via two accumulating matmuls
  with host-precomputed block-diagonal weights.  SBUF layout: partitions
  hold (d,no) for embeddings / (l,no) for masks, where no indexes 8 strips
  of N (each partition's data is a contiguous DRAM run -> fast DMA).
  Then dist=sqrt(d2+m_sq) (ACT), masked hinge + square + fused reduce (DVE).
Host (numpy): segment means/counts (tiny O(B*L*D)), pairwise push loss over
  centroids, final scalar assembly.  Scalar losses summed over cores on host.
"""

import sys

sys.path.insert(0, "/opt/trn_rl_repo")

import numpy as np

EMBED_DIM = 16
DELTA_V = 0.5
DELTA_D = 3.0

B, D, N = 16, 16, 80000
L = 8
NCORES = 8
BPC = B // NCORES          # batches per core = 2
NO = 8                     # strips along N
NI = N // NO               # 10000 elements per strip per partition-row
CHUNK = 2000               # ni-chunk per pipeline step
NCHUNK = NI // CHUNK       # 5 chunks per batch
MMF = 500                  # matmul free dim (<=512 f32 per PSUM bank)
NMM = CHUNK // MMF         # 4 matmul pairs per chunk

_CACHE = {}


def _build_module():
    import concourse.bass as bass
    import concourse.mybir as mybir
    import concourse.tile as tile

    f32 = mybir.dt.float32
    bf16 = mybir.dt.bfloat16
    i32 = mybir.dt.int32

    nc = bass.Bass()
    emb_d = nc.dram_tensor("emb", [BPC, D, N], f32, kind="ExternalInput")
    seg_d = nc.dram_tensor("seg", [BPC, L, N], i32, kind="ExternalInput")
    wall_d = nc.dram_tensor("wall", [128, 1 + 2 * BPC, 64], bf16, kind="ExternalInput")
    out_d = nc.dram_tensor("out", [BPC, 64], f32, kind="ExternalOutput")

    with tile.TileContext(nc) as tc:
        with (
            tc.tile_pool(name="consts", bufs=1) as cpool,
            tc.tile_pool(name="work", bufs=3) as pool,
            tc.tile_pool(name="acc", bufs=1) as apool,
            tc.tile_pool(name="psum", bufs=2, space="PSUM") as ppool,
        ):
            # packed weights: [:,0]=w2, [:,1+b]=wm_b, [:,1+BPC+b]=wq_b
            wall = cpool.tile([128, 1 + 2 * BPC, 64], bf16)
            nc.sync.dma_start(wall[:], wall_d[:])
            ones = cpool.tile([128, MMF], bf16)
            nc.vector.memset(ones[:], 1.0)
            acc = apool.tile([64, BPC, NCHUNK], f32)
            pullsum = apool.tile([64, BPC], f32)

            emb_v = emb_d[:].rearrange("b d (no ni) -> (d no) b ni", no=NO)
            seg_v = seg_d[:].rearrange("b l (no ni) -> (l no) b ni", no=NO)

            # single cast-DMAs for the whole kernel (2 SWDGE lanes total;
            # unique tiles -> the DMAs carry zero sem waits)
            ea = pool.tile([128, BPC, NI], bf16, tag="ea", bufs=1)
            nc.gpsimd.dma_start(ea[:], emb_v)          # f32 -> bf16
            ma = pool.tile([64, BPC, NI], bf16, tag="ma", bufs=1)
            nc.gpsimd.dma_start(ma[:], seg_v)          # i32 -> bf16 (0/1)
            # absorber: advance DVE's observed clock past the mask DMA so
            # per-chunk mask multiplies don't need a second sem wait
            dummy = pool.tile([64, 1], bf16, tag="dummy")
            nc.vector.tensor_copy(dummy[:], ma[:, :1, 0])

            for b in range(BPC):
                e = ea[:, b, :]
                mask = ma[:, b, :]
                for c in range(NCHUNK):
                    e2 = pool.tile([128, CHUNK], bf16, tag=f"e2_{b}_{c}", bufs=1)
                    d2 = ppool.tile([64, NMM, 512], f32, tag="d2")
                    for j in range(NMM):
                        o = c * CHUNK + j * MMF
                        jsl = slice(o, o + MMF)
                        nc.tensor.matmul(
                            d2[:, j, :MMF], wall[:, 1 + BPC + b, :], ones[:],
                            start=True, stop=False,
                        )
                        nc.tensor.matmul(
                            d2[:, j, :MMF], wall[:, 1 + b, :], e[:, jsl],
                            start=False, stop=False,
                        )
                        nc.vector.tensor_tensor(
                            e2[:, j * MMF : (j + 1) * MMF],
                            e[:, jsl], e[:, jsl], mybir.AluOpType.mult,
                        )
                        nc.tensor.matmul(
                            d2[:, j, :MMF], wall[:, 0, :],
                            e2[:, j * MMF : (j + 1) * MMF],
                            start=False, stop=True,
                        )
                    dist = pool.tile([64, CHUNK], bf16, tag=f"dist_{b}_{c}", bufs=1)
                    for j in range(NMM):
                        nc.scalar.activation(
                            dist[:, j * MMF : (j + 1) * MMF],
                            d2[:, j, :MMF],
                            mybir.ActivationFunctionType.Sqrt,
                        )
                    # hinge: v = max(s, dv) - dv  == relu(s - dv), in place
                    nc.vector.tensor_scalar(
                        dist[:], dist[:], DELTA_V, -DELTA_V,
                        mybir.AluOpType.max, mybir.AluOpType.add,
                    )
                    # apply 0/1 mask, in place
                    nc.vector.tensor_tensor(
                        dist[:], dist[:],
                        mask[:, c * CHUNK : (c + 1) * CHUNK],
                        mybir.AluOpType.mult,
                    )
                    # square + free-dim sum in one fused op
                    nc.vector.scalar_tensor_tensor(
                        dist[:], dist[:], 1.0, dist[:],
                        mybir.AluOpType.mult, mybir.AluOpType.mult,
                        accum_out=acc[:, b, c : c + 1],
                    )
                nc.vector.tensor_reduce(
                    pullsum[:, b : b + 1],
                    acc[:, b, :],
                    axis=mybir.AxisListType.X,
                    op=mybir.AluOpType.add,
                )
            nc.sync.dma_start(out_d[:].rearrange("b p -> p b"), pullsum[:])
    return nc


def _get_nc():
    if "nc" not in _CACHE:
        _CACHE["nc"] = _build_module()
    return _CACHE["nc"]


def _to_bf16(x):
    import jax.numpy as jnp

    return np.asarray(jnp.asarray(np.asarray(x, np.float32), dtype=jnp.bfloat16))


def run_device(embedding, seg_gt, means, m_sq):
    from concourse.bass_utils import run_bass_kernel_spmd

    nc = _get_nc()
    # block-diagonal selector: w2[(d,no),(l,no')] = (no==no')
    eye_no = np.eye(NO, dtype=np.float32)
    w2 = np.tile(eye_no, (D, L))  # wrong layout; build explicitly instead
    w2 = np.zeros((128, 64), np.float32)
    w2 = (
        np.ones((D, 1, L, 1), np.float32) * eye_no[None, :, None, :]
    ).reshape(128, 64)

    in_maps = []
    for cid in range(NCORES):
        b0 = cid * BPC
        # wm[(d,no),(l,no')] = -2*means[b,l,d] * (no==no')
        m = means[b0 : b0 + BPC]  # [BPC, L, D]
        wm = (
            -2.0
            * m.transpose(0, 2, 1)[:, :, None, :, None]  # [BPC, D, 1, L, 1]
            * eye_no[None, None, :, None, :]
        ).reshape(BPC, 128, 64)
        # wq[(d,no),(l,no')] = m_sq[b,l]/D * (no==no'); contracted against a
        # ones rhs this adds m_sq[l] into every d2 element of lane l.
        wq = np.broadcast_to(
            (m_sq[b0 : b0 + BPC] / float(D))[:, None, None, :, None]
            * eye_no[None, None, :, None, :],
            (BPC, D, NO, L, NO),
        ).reshape(BPC, 128, 64)
        wall = np.zeros((128, 1 + 2 * BPC, 64), np.float32)
        wall[:, 0, :] = w2
        for bb in range(BPC):
            wall[:, 1 + bb, :] = wm[bb]
            wall[:, 1 + BPC + bb, :] = wq[bb]
        in_maps.append(
            {
                "emb": np.ascontiguousarray(embedding[b0 : b0 + BPC]),
                "seg": np.ascontiguousarray(seg_gt[b0 : b0 + BPC]),
                "wall": _to_bf16(wall),
            }
        )
    res = run_bass_kernel_spmd(nc, in_maps, core_ids=list(range(NCORES)))
    _CACHE["last_exec_ns"] = res.exec_time_ns
    pull = np.zeros((B, L), np.float32)
    for cid in range(NCORES):
        o = np.asarray(res.results[cid]["out"], np.float32)  # [BPC, 64]
        for bb in range(BPC):
            pull[cid * BPC + bb] = o[bb].reshape(L, NO).sum(axis=1)
    return pull


def kernel(embedding, seg_gt):
    embedding = np.asarray(embedding, np.float32)
    seg_gt = np.asarray(seg_gt, np.int32)

    maskf = (seg_gt > 0).astype(np.float32)          # [B, L, N]
    counts = maskf.sum(-1)                            # [B, L]
    valid = counts > 0
    cnt_safe = np.maximum(counts, 1.0)
    # means via per-batch sgemm: [L,N] @ [N,D]
    sums = np.einsum("bln,bdn->bld", maskf, embedding, optimize=True)
    means = (sums / cnt_safe[..., None]).astype(np.float32)  # [B, L, D]
    m_sq = (means * means).sum(-1)                    # [B, L]

    try:
        pull_sums = run_device(embedding, seg_gt, means, m_sq)  # [B, L]
    except Exception:
        import traceback; traceback.print_exc()
        pull_sums = _pull_sums_jax(embedding, seg_gt, means, m_sq)

    lane_mean = pull_sums / cnt_safe
    var_loss = np.where(valid, lane_mean, 0.0).sum() / L / B

    # push (distance) loss on host from centroids
    diffc = means[:, :, None, :] - means[:, None, :, :]
    d2c = (diffc * diffc).sum(-1)
    pos = d2c > 0
    distc = np.where(pos, np.sqrt(np.where(pos, d2c, 1.0)), 0.0)
    distc = distc + np.eye(L, dtype=np.float32)[None] * DELTA_D
    pair_valid = (valid[:, :, None] & valid[:, None, :]).astype(np.float32)
    hinge = np.maximum(DELTA_D - distc, 0.0) ** 2 * pair_valid
    nv = valid.sum(-1).astype(np.float32)
    denom = nv * (nv - 1.0)
    per_b = np.where(
        nv > 1.0, hinge.sum(axis=(1, 2)) / np.maximum(denom, 1.0) / 2.0, 0.0
    )
    dist_loss = per_b.sum() / B

    return (
        np.float32(var_loss),
        np.float32(dist_loss),
        np.zeros((), np.float32),
    )


def _pull_sums_jax(embedding, seg_gt, means, m_sq):
    """Fallback: compute masked pull sums on the NeuronCores via jax/XLA,
    sharded over batch."""
    import jax, jax.numpy as jnp
    devs = jax.devices()[:NCORES]

    def one_shard(emb, seg, mn, msq):
        maskf = (seg > 0).astype(jnp.float32)
        e_sq = jnp.sum(emb * emb, axis=1)
        cross = jnp.einsum("bld,bdn->bln", mn, emb)
        d2 = jnp.maximum(e_sq[:, None, :] - 2.0 * cross + msq[..., None], 0.0)
        dist = jnp.sqrt(d2)
        pull = jnp.maximum(dist - DELTA_V, 0.0) ** 2
        return jnp.sum(maskf * pull, axis=-1)

    f = jax.jit(one_shard)
    outs = []
    for cid in range(NCORES):
        b0 = cid * BPC
        args = [jax.device_put(np.ascontiguousarray(x), devs[cid]) for x in
                (embedding[b0:b0+BPC], seg_gt[b0:b0+BPC],
                 means[b0:b0+BPC], m_sq[b0:b0+BPC])]
        outs.append(f(*args))
    return np.concatenate([np.asarray(o) for o in outs], axis=0)


# revision 19
# speedup vs baseline: 2.0994x; 2.0994x over previous
"""Discriminative loss kernel for Trainium2 (8 NeuronCores, batch-parallel).

Device kernel (per core, 2 batches): the O(B*L*N) "pull" pass.
  d2[l,n] = e_sq[n] - 2*cross[l,n] + m_sq[l] via two accumulating matmuls
  with host-precomputed block-diagonal weights.  SBUF layout: partitions
  hold (d,no) for embeddings / (l,no) for masks, where no indexes 8 strips
  of N (each partition's data is a contiguous DRAM run -> fast DMA).
  Then dist=sqrt(d2+m_sq) (ACT), masked hinge + square + fused reduce (DVE).
Host (numpy): segment means/counts (tiny O(B*L*D)), pairwise push loss over
  centroids, final scalar assembly.  Scalar losses summed over cores on host.
"""

import sys

sys.path.insert(0, "/opt/trn_rl_repo")

import numpy as np

EMBED_DIM = 16
DELTA_V = 0.5
DELTA_D = 3.0

B, D, N = 16, 16, 80000
L = 8
NCORES = 8
BPC = B // NCORES          # batches per core = 2
NO = 8                     # strips along N
NI = N // NO               # 10000 elements per strip per partition-row
CHUNK = 2000               # ni-chunk per pipeline step
NCHUNK = NI // CHUNK       # 5 chunks per batch
MMF = 500                  # matmul free dim (<=512 f32 per PSUM bank)
NMM = CHUNK // MMF         # 4 matmul pairs per chunk

_CACHE = {}


def _build_module():
    import concourse.bass as bass
    import concourse.mybir as mybir
    import concourse.tile as tile

    f32 = mybir.dt.float32
    bf16 = mybir.dt.bfloat16
    i32 = mybir.dt.int32

    nc = bass.Bass()
    emb_d = nc.dram_tensor("emb", [BPC, D, N], f32, kind="ExternalInput")
    seg_d = nc.dram_tensor("seg", [BPC, L, N], i32, kind="ExternalInput")
    wall_d = nc.dram_tensor("wall", [128, 1 + 2 * BPC, 64], bf16, kind="ExternalInput")
    out_d = nc.dram_tensor("out", [BPC, 64], f32, kind="ExternalOutput")

    with tile.TileContext(nc) as tc:
        with (
            tc.tile_pool(name="consts", bufs=1) as cpool,
            tc.tile_pool(name="work", bufs=3) as pool,
            tc.tile_pool(name="acc", bufs=1) as apool,
            tc.tile_pool(name="psum", bufs=2, space="PSUM") as ppool,
        ):
            # packed weights: [:,0]=w2, [:,1+b]=wm_b, [:,1+BPC+b]=wq_b
            wall = cpool.tile([128, 1 + 2 * BPC, 64], bf16)
            nc.sync.dma_start(wall[:], wall_d[:])
            ones = cpool.tile([128, MMF], bf16)
            nc.vector.memset(ones[:], 1.0)
            acc = apool.tile([64, BPC, NCHUNK], f32)
            pullsum = apool.tile([64, BPC], f32)

            emb_v = emb_d[:].rearrange("b d (no ni) -> (d no) b ni", no=NO)
            seg_v = seg_d[:].rearrange("b l (no ni) -> (l no) b ni", no=NO)

            # single cast-DMAs for the whole kernel (2 SWDGE lanes total;
            # unique tiles -> the DMAs carry zero sem waits)
            ea = pool.tile([128, BPC, NI], bf16, tag="ea", bufs=1)
            nc.gpsimd.dma_start(ea[:], emb_v)          # f32 -> bf16
            ma = pool.tile([64, BPC, NI], bf16, tag="ma", bufs=1)
            nc.gpsimd.dma_start(ma[:], seg_v)          # i32 -> bf16 (0/1)
            # absorber: advance DVE's observed clock past the mask DMA so
            # per-chunk mask multiplies don't need a second sem wait
            dummy = pool.tile([64, 1], bf16, tag="dummy")
            nc.vector.tensor_copy(dummy[:], ma[:, :1, 0])

            for b in range(BPC):
                e = ea[:, b, :]
                mask = ma[:, b, :]
                for c in range(NCHUNK):
                    e2 = pool.tile([128, CHUNK], bf16, tag=f"e2_{b}_{c}", bufs=1)
                    d2 = ppool.tile([64, NMM, 512], f32, tag="d2")
                    for j in range(NMM):
                        o = c * CHUNK + j * MMF
                        jsl = slice(o, o + MMF)
                        nc.tensor.matmul(
                            d2[:, j, :MMF], wall[:, 1 + BPC + b, :], ones[:],
                            start=True, stop=False,
                        )
                        nc.tensor.matmul(
                            d2[:, j, :MMF], wall[:, 1 + b, :], e[:, jsl],
                            start=False, stop=False,
                        )
                        nc.vector.tensor_tensor(
                            e2[:, j * MMF : (j + 1) * MMF],
                            e[:, jsl], e[:, jsl], mybir.AluOpType.mult,
                        )
                        nc.tensor.matmul(
                            d2[:, j, :MMF], wall[:, 0, :],
                            e2[:, j * MMF : (j + 1) * MMF],
                            start=False, stop=True,
                        )
                    dist = pool.tile([64, CHUNK], bf16, tag=f"dist_{b}_{c}", bufs=1)
                    for j in range(NMM):
                        nc.scalar.activation(
                            dist[:, j * MMF : (j + 1) * MMF],
                            d2[:, j, :MMF],
                            mybir.ActivationFunctionType.Sqrt,
                        )
                    # hinge: v = max(s, dv) - dv  == relu(s - dv), in place
                    nc.vector.tensor_scalar(
                        dist[:], dist[:], DELTA_V, -DELTA_V,
                        mybir.AluOpType.max, mybir.AluOpType.add,
                    )
                    # apply 0/1 mask, in place
                    nc.vector.tensor_tensor(
                        dist[:], dist[:],
                        mask[:, c * CHUNK : (c + 1) * CHUNK],
                        mybir.AluOpType.mult,
                    )
                    # square + free-dim sum in one fused op
                    nc.vector.scalar_tensor_tensor(
                        dist[:], dist[:], 1.0, dist[:],
                        mybir.AluOpType.mult, mybir.AluOpType.mult,
                        accum_out=acc[:, b, c : c + 1],
                    )
                nc.vector.tensor_reduce(
                    pullsum[:, b : b + 1],
                    acc[:, b, :],
                    axis=mybir.AxisListType.X,
                    op=mybir.AluOpType.add,
                )
            nc.sync.dma_start(out_d[:].rearrange("b p -> p b"), pullsum[:])
    return nc


def _get_nc():
    if "nc" not in _CACHE:
        _CACHE["nc"] = _build_module()
    return _CACHE["nc"]


def _to_bf16(x):
    import jax.numpy as jnp

    return np.asarray(jnp.asarray(np.asarray(x, np.float32), dtype=jnp.bfloat16))


def run_device(embedding, seg_gt, means, m_sq):
    from concourse.bass_utils import run_bass_kernel_spmd

    nc = _get_nc()
    # block-diagonal selector: w2[(d,no),(l,no')] = (no==no')
    eye_no = np.eye(NO, dtype=np.float32)
    w2 = np.tile(eye_no, (D, L))  # wrong layout; build explicitly instead
    w2 = np.zeros((128, 64), np.float32)
    w2 = (
        np.ones((D, 1, L, 1), np.float32) * eye_no[None, :, None, :]
    ).reshape(128, 64)

    in_maps = []
    for cid in range(NCORES):
        b0 = cid * BPC
        # wm[(d,no),(l,no')] = -2*means[b,l,d] * (no==no')
        m = means[b0 : b0 + BPC]  # [BPC, L, D]
        wm = (
            -2.0
            * m.transpose(0, 2, 1)[:, :, None, :, None]  # [BPC, D, 1, L, 1]
            * eye_no[None, None, :, None, :]
        ).reshape(BPC, 128, 64)
        # wq[(d,no),(l,no')] = m_sq[b,l]/D * (no==no'); contracted against a
        # ones rhs this adds m_sq[l] into every d2 element of lane l.
        wq = np.broadcast_to(
            (m_sq[b0 : b0 + BPC] / float(D))[:, None, None, :, None]
            * eye_no[None, None, :, None, :],
            (BPC, D, NO, L, NO),
        ).reshape(BPC, 128, 64)
        wall = np.zeros((128, 1 + 2 * BPC, 64), np.float32)
        wall[:, 0, :] = w2
        for bb in range(BPC):
            wall[:, 1 + bb, :] = wm[bb]
            wall[:, 1 + BPC + bb, :] = wq[bb]
        in_maps.append(
            {
                "emb": np.ascontiguousarray(embedding[b0 : b0 + BPC]),
                "seg": np.ascontiguousarray(seg_gt[b0 : b0 + BPC]),
                "wall": _to_bf16(wall),
            }
        )
    res = run_bass_kernel_spmd(nc, in_maps, core_ids=list(range(NCORES)))
    _CACHE["last_exec_ns"] = res.exec_time_ns
    pull = np.zeros((B, L), np.float32)
    for cid in range(NCORES):
        o = np.asarray(res.results[cid]["out"], np.float32)  # [BPC, 64]
        for bb in range(BPC):
            pull[cid * BPC + bb] = o[bb].reshape(L, NO).sum(axis=1)
    return pull


def kernel(embedding, seg_gt):
    embedding = np.asarray(embedding, np.float32)
    seg_gt = np.asarray(seg_gt, np.int32)

    maskf = (seg_gt > 0).astype(np.float32)          # [B, L, N]
    counts = maskf.sum(-1)                            # [B, L]
    valid = counts > 0
    cnt_safe = np.maximum(counts, 1.0)
    # means via per-batch sgemm: [L,N] @ [N,D]
    sums = np.einsum("bln,bdn->bld", maskf, embedding, optimize=True)
    means = (sums / cnt_safe[..., None]).astype(np.float32)  # [B, L, D]
    m_sq = (means * means).sum(-1)                    # [B, L]

    pull_sums = _pull_sums_jax(embedding, seg_gt, means, m_sq)

    lane_mean = pull_sums / cnt_safe
    var_loss = np.where(valid, lane_mean, 0.0).sum() / L / B

    # push (distance) loss on host from centroids
    diffc = means[:, :, None, :] - means[:, None, :, :]
    d2c = (diffc * diffc).sum(-1)
    pos = d2c > 0
    distc = np.where(pos, np.sqrt(np.where(pos, d2c, 1.0)), 0.0)
    distc = distc + np.eye(L, dtype=np.float32)[None] * DELTA_D
    pair_valid = (valid[:, :, None] & valid[:, None, :]).astype(np.float32)
    hinge = np.maximum(DELTA_D - distc, 0.0) ** 2 * pair_valid
    nv = valid.sum(-1).astype(np.float32)
    denom = nv * (nv - 1.0)
    per_b = np.where(
        nv > 1.0, hinge.sum(axis=(1, 2)) / np.maximum(denom, 1.0) / 2.0, 0.0
    )
    dist_loss = per_b.sum() / B

    return (
        np.float32(var_loss),
        np.float32(dist_loss),
        np.zeros((), np.float32),
    )


def _pull_sums_jax(embedding, seg_gt, means, m_sq):
    """Masked pull sums on the 8 NeuronCores via one pmap, batch-sharded."""
    import jax, jax.numpy as jnp

    def one_shard(emb, seg, mn, msq):
        maskf = (seg > 0).astype(jnp.float32)
        e_sq = jnp.sum(emb * emb, axis=1)
        cross = jnp.einsum("bld,bdn->bln", mn, emb)
        d2 = jnp.maximum(e_sq[:, None, :] - 2.0 * cross + msq[..., None], 0.0)
        dist = jnp.sqrt(d2)
        pull = jnp.maximum(dist - DELTA_V, 0.0) ** 2
        return jnp.sum(maskf * pull, axis=-1)

    f = jax.pmap(one_shard)
    sh = lambda x: np.ascontiguousarray(x).reshape(NCORES, BPC, *x.shape[1:])
    out = f(sh(embedding), sh(seg_gt), sh(means), sh(m_sq))
    return np.asarray(out).reshape(B, L)
